# revision 26
# baseline (speedup 1.0000x reference)
"""CQT (constant-Q transform) kernel for Trainium2, 8 NeuronCores.

Math: out[b, c, t] = sum_l W[c, l] * x_pad[b, t*HOP + l]   (strided conv,
HOP=512, L=11339 taps, C=168 channels = 84 bins x re/im), then reshaped to
(B, 2, n_bins, T_out).

Strategy:
  - Data-parallel: shard B=32 across 8 cores (4 batches/core), weights
    replicated.
  - The conv is decomposed into 128-tap blocks: block p covers taps
    [128p, 128p+128).  The moving operand for block p=(4j+k) at output
    tile [t0, t0+nt) is a contiguous column slice of a host-pre-transposed
    view of x:  xt[r, k, u] = x_pad[512u + 128k + r].
  - CQT kernels are ragged (bin k has ~11339*2^(-k/12) taps, centered), so
    most blocks touch only a few low-bin channels.  A plain matmul costs
    ~N streaming cycles regardless of how few of the 128 PE columns hold
    weights, so the dense-block formulation wastes most of the array.
  - Column tiling: channels are split into groups of 32 (16 bins).  Each
    (block, group) quantum is a K=128, M<=32, N=nt matmul placed on one of
    the four 32-column PE tile positions (tile_position=(0, 32*slot)).
    The 4 tile positions stream concurrently, quartering PE time.
    Quanta per t-tile per group: {89, 36, 15, 7, 3, 2} = 152 vs 92
    full-width matmuls for the dense-block formulation; packed on 4 slots
    the makespan is 114 passes/batch vs 276 -> ~2.4x less PE streaming.
  - Each (group, t-tile) job accumulates its blocks into its own PSUM bank
    (per-element has_written semantics: first write overwrites, later ones
    accumulate), then DVE-copies psum[32s:32s+m] -> SBUF and DMAs to out.
    Static balanced schedule: 4 slots x 114 passes per batch.
"""

import numpy as np

HOP = 512
N_CORES = 8

_prog_cache: dict = {}


def _host_prep(x, kernels):
    x = np.ascontiguousarray(np.asarray(x, dtype=np.float32))
    kernels = np.ascontiguousarray(np.asarray(kernels, dtype=np.float32))
    B, T = x.shape
    nbins, two, Lmax = kernels.shape
    assert two == 2
    C = 2 * nbins
    pad = Lmax // 2
    T_out = (T + 2 * pad - Lmax) // HOP + 1

    # ---- weights: pad taps to 128 multiple ----
    nblk = -(-Lmax // 128)
    Wp = np.zeros((C, nblk * 128), dtype=np.float32)
    Wp[:, :Lmax] = kernels.reshape(C, Lmax)
    nzb = (Wp.reshape(C, nblk, 128) != 0.0).any(axis=2)  # [C, nblk]

    # channel groups of 32 (16 bins); bins are sorted by descending filter
    # length, supports are nested, so a group's active blocks = union over
    # its channels = the blocks of its longest (first) channel.
    groups = []  # (c0, m, blocks)
    for c0 in range(0, C, 32):
        m = min(32, C - c0)
        blks = np.where(nzb[c0:c0 + m].any(axis=0))[0].tolist()
        groups.append((c0, m, blks))

    # Weight layout: per (group, block) a zero-padded [128 taps, m chans]
    # panel.  Panels are laid out in CONSUMPTION order: wave i holds panel
    # i of every group that still has blocks (all slots consume their
    # group's panels in lockstep), so a prefix of wt's columns is exactly
    # the first waves -- weight DMA chunks can be few and large.
    maxlen_b = max(len(b) for _, _, b in groups)
    wpos = [[None] * len(blks) for _, _, blks in groups]
    wave_col = []  # column where wave i starts
    tot = 0
    for i in range(maxlen_b):
        wave_col.append(tot)
        for g, (c0, m, blks) in enumerate(groups):
            if i < len(blks):
                wpos[g][i] = tot
                tot += m
    wave_col.append(tot)
    wt = np.zeros((128, tot), dtype=np.float32)
    for g, (c0, m, blks) in enumerate(groups):
        for rel, p in enumerate(blks):
            w0 = wpos[g][rel]
            wt[:, w0: w0 + m] = Wp[c0:c0 + m, 128 * p: 128 * (p + 1)].T
    import ml_dtypes
    wt = np.ascontiguousarray(wt.astype(ml_dtypes.bfloat16))

    # ---- x: pad and pre-transpose to [128, 4, U] per batch ----
    j_max = (nblk - 1) // 4
    U = T_out + j_max
    xpad_len = 512 * U
    assert xpad_len >= pad + T, (xpad_len, pad + T)
    xp = np.zeros((B, xpad_len), dtype=np.float32)
    xp[:, pad:pad + T] = x
    # xt[b, r, k*U + u] = xp[b, 512u + 128k + r]
    import ml_dtypes
    xt = np.ascontiguousarray(
        xp.reshape(B, U, 4, 128).transpose(0, 3, 2, 1).reshape(B, 128, 4 * U)
        .astype(ml_dtypes.bfloat16)
    )
    return xt, wt, groups, (wpos, wave_col), C, U, T_out, nbins


def _build_schedule(groups, T_out, b_per):
    """Static balanced 4-slot schedule over the WHOLE core (no per-batch
    barriers).  Jobs are (batch b, group g, t-tile tt); job = len(blocks)
    passes.  Per batch the slot loads are 114/114/114/114; slots flow
    straight from one batch into the next, so PSUM/eviction reuse never
    synchronizes across slots."""
    nts = []
    t0 = 0
    while t0 < T_out:
        nts.append((t0, min(512, T_out - t0)))
        t0 += 512
    assert len(nts) == 3 and len(groups) == 6
    SLOT_JOBS = [
        [(0, 0), (2, 2), (3, 2), (4, 2)],
        [(1, 0), (1, 1), (1, 2), (5, 0), (5, 1), (5, 2)],
        [(2, 0), (3, 0), (4, 0), (0, 1)],
        [(2, 1), (3, 1), (4, 1), (0, 2)],
    ]
    # coverage check
    seen = set()
    for sj in SLOT_JOBS:
        for g, tt in sj:
            assert (g, tt) not in seen
            seen.add((g, tt))
    assert seen == {(g, tt) for g in range(6) for tt in range(3)}

    # flatten to per-slot quanta across all batches
    slot_q = []
    for sj in SLOT_JOBS:
        qs = []
        for b in range(b_per):
            for g, tt in sj:
                blks = groups[g][2]
                for rel, p in enumerate(blks):
                    qs.append(
                        (b, g, tt, p, rel, rel == 0, rel == len(blks) - 1)
                    )
        slot_q.append(qs)
    return nts, slot_q


def _build_program(b_per, C, U, T_out, groups, wmeta):
    import concourse.mybir as mybir
    import concourse.tile as tile
    from concourse import bacc

    f32 = mybir.dt.float32
    bf16 = mybir.dt.bfloat16
    wpos, wave_col = wmeta
    wtot = wave_col[-1]
    nts, slot_q = _build_schedule(groups, T_out, b_per)
    maxlen = max(len(q) for q in slot_q)

    nc = bacc.Bacc(
        "TRN2",
        target_bir_lowering=False,
        debug=False,
        enable_asserts=True,
        num_devices=N_CORES,
    )
    xt_d = nc.dram_tensor("xt", [b_per, 128, 4 * U], bf16, kind="ExternalInput").ap()
    wt_d = nc.dram_tensor("wt", [128, wtot], bf16, kind="ExternalInput").ap()
    out_d = nc.dram_tensor("out", [b_per, C, T_out], f32, kind="ExternalOutput").ap()

    # weight DMA chunk boundaries: waves 0-23, 24-47, 48+ (large
    # contiguous transfers in consumption order)
    nwave = len(wave_col) - 1
    w_cuts = [0] + [wave_col[min(w, nwave)] for w in (24, 48)] + [wtot]
    w_cuts = sorted(set(w_cuts))

    # x DMA chunk boundaries for batch 0 (u-ranges per t-tile window)
    j_max = (max(groups[0][2])) // 4
    x_stops = []
    for (t0_, nt_) in nts:
        x_stops.append(min(t0_ + nt_ + j_max + 1, U))
    x_stops[-1] = U

    with tile.TileContext(nc) as tc:
        with (
            tc.tile_pool(name="wpool", bufs=1) as wpool,
            tc.tile_pool(name="xpool", bufs=4) as xpool,
            tc.tile_pool(name="evpool", bufs=6) as evpool,
            tc.tile_pool(name="pspool", bufs=8, space="PSUM") as pspool,
        ):
            wsb = wpool.tile([128, wtot], bf16)
            xbs = [
                xpool.tile([128, 4 * U], bf16, tag="xb", name=f"xb{b}")
                for b in range(b_per)
            ]

            # PE warmup: the HAM clock gate keeps the PE at 1.2 GHz until
            # it has been busy ~3.4us.  Burn that in on scratch data while
            # the first input DMAs are in flight so the real matmuls run
            # at 2.4 GHz from pass 0.
            wu_rhs = wpool.tile([128, 512], bf16, name="wu_rhs")
            wu_ps = pspool.tile([128, 512], f32, tag="ps", name="wu_ps")
            nc.vector.memset(wu_rhs[:], 0.0)
            for _ in range(18):
                nc.tensor.matmul(
                    wu_ps[0:32, :],
                    lhsT=wu_rhs[:, 0:32],
                    rhs=wu_rhs[:],
                    start=True,
                    stop=True,
                    tile_position=(0, 0),
                )

            src0 = xt_d[0].rearrange("r (k u) -> r k u", k=4)
            dst0 = xbs[0].rearrange("r (k u) -> r k u", k=4)
            u2 = x_stops[1]
            # scalar queue: weights in 3 large wave-ordered transfers
            for a0, a1 in zip(w_cuts[:-1], w_cuts[1:]):
                nc.scalar.dma_start(out=wsb[:, a0:a1], in_=wt_d[:, a0:a1])
            # sync queue: one big x transfer covering every t-tile window
            # of batch 0, the short tail window, then batch prefetches
            nc.sync.dma_start(out=dst0[:, :, 0:u2], in_=src0[:, :, 0:u2])
            nc.sync.dma_start(out=dst0[:, :, u2:U], in_=src0[:, :, u2:U])
            for b in range(1, b_per):
                nc.sync.dma_start(out=xbs[b][:], in_=xt_d[b])

            # one global pass loop — slots flow across batch boundaries;
            # PSUM banks and eviction buffers rotate through shared pools
            # so reuse distance is ~8 jobs (many microseconds of slack).
            cur_ps = [None] * 4
            for i in range(maxlen):
                for s in range(4):
                    if i >= len(slot_q[s]):
                        continue
                    b, g, tt, p, rel, first, last = slot_q[s][i]
                    c0, m, blks = groups[g]
                    t0, nt = nts[tt]
                    xb = xbs[b]
                    if first:
                        cur_ps[s] = pspool.tile(
                            [128, 512], f32, tag="ps", name=f"ps{s}_{b}_{g}_{tt}"
                        )
                    ps = cur_ps[s]
                    j, k = divmod(p, 4)
                    wc = wpos[g][rel]
                    nc.tensor.matmul(
                        ps[32 * s: 32 * s + m, :nt],
                        lhsT=wsb[:, wc: wc + m],
                        rhs=xb[:, k * U + t0 + j: k * U + t0 + j + nt],
                        start=first,
                        stop=last,
                        tile_position=(0, 32 * s),
                    )
                    if last:
                        ev = evpool.tile(
                            [128, 512], f32, tag="ev", name=f"ev{s}_{b}_{g}_{tt}"
                        )
                        if s < 2:
                            nc.vector.tensor_copy(
                                ev[32 * s: 32 * s + m, :nt],
                                ps[32 * s: 32 * s + m, :nt],
                            )
                            nc.sync.dma_start(
                                out=out_d[b, c0:c0 + m, t0:t0 + nt],
                                in_=ev[32 * s: 32 * s + m, :nt],
                            )
                        else:
                            nc.scalar.copy(
                                ev[32 * s: 32 * s + m, :nt],
                                ps[32 * s: 32 * s + m, :nt],
                            )
                            nc.scalar.dma_start(
                                out=out_d[b, c0:c0 + m, t0:t0 + nt],
                                in_=ev[32 * s: 32 * s + m, :nt],
                            )
    nc.compile()
    return nc


def _ensure_trace_shims():
    """If run_bass_kernel_spmd is invoked with tracing enabled (e.g. via
    BASS_TRACE=1) it imports antenv.axon_hooks and uploads artifacts to a
    bucket; neither exists in a bare container.  Register a working NTFF
    hook (ctypes into the axon .so) and a no-op uploader so the trace path
    degrades gracefully instead of crashing."""
    import sys

    try:
        import antenv.axon_hooks  # noqa: F401
    except ImportError:
        import contextlib
        import ctypes
        import types

        hook = None
        try:
            lib = ctypes.CDLL("/opt/axon/libaxon_pjrt.so")
            if hasattr(lib, "axon_start_nrt_profile"):
                lib.axon_start_nrt_profile.argtypes = [
                    ctypes.POINTER(ctypes.c_int64),
                    ctypes.c_size_t,
                ]
                lib.axon_start_nrt_profile.restype = ctypes.c_int64
                lib.axon_stop_nrt_profile.argtypes = [ctypes.c_char_p]
                lib.axon_stop_nrt_profile.restype = ctypes.c_int64

                @contextlib.contextmanager
                def _hook(output_dir, device_ids):
                    import jax

                    jax.devices()
                    if device_ids:
                        ids = (ctypes.c_int64 * len(device_ids))(*device_ids)
                        rc = lib.axon_start_nrt_profile(ids, len(device_ids))
                    else:
                        rc = lib.axon_start_nrt_profile(None, 0)
                    if rc != 0:
                        raise RuntimeError(f"axon_start_nrt_profile rc={rc}")
                    try:
                        yield
                    finally:
                        lib.axon_stop_nrt_profile(str(output_dir).encode())

                hook = _hook
        except OSError:
            pass
        mod = types.ModuleType("antenv.axon_hooks")
        mod.get_axon_ntff_profile_hook = lambda: hook
        mod.set_axon_ntff_profile_hook = lambda h: None
        sys.modules["antenv.axon_hooks"] = mod

    try:
        import concourse.bass_utils as _bu

        _orig_upload = _bu.upload_artifacts

        def _safe_upload(tmpdir):
            try:
                return _orig_upload(tmpdir)
            except Exception:
                return "local://unavailable"

        if not getattr(_bu, "_safe_upload_installed", False):
            _bu.upload_artifacts = _safe_upload
            _bu._safe_upload_installed = True
    except Exception:
        pass


def kernel(x, kernels):
    _ensure_trace_shims()
    from concourse.bass_utils import run_bass_kernel_spmd

    xt, wt, groups, wmeta, C, U, T_out, nbins = _host_prep(x, kernels)
    B = xt.shape[0]
    assert B % N_CORES == 0
    b_per = B // N_CORES

    key = (b_per, C, U, T_out, tuple((c0, m, tuple(b)) for c0, m, b in groups))
    if key not in _prog_cache:
        _prog_cache[key] = _build_program(b_per, C, U, T_out, groups, wmeta)
    nc = _prog_cache[key]

    in_maps = [
        {"xt": xt[c * b_per:(c + 1) * b_per], "wt": wt} for c in range(N_CORES)
    ]
    res = run_bass_kernel_spmd(nc, in_maps, list(range(N_CORES)))
    parts = [res.results[c]["out"] for c in range(N_CORES)]
    out = np.concatenate(parts, axis=0)  # (B, C, T_out)
    return np.ascontiguousarray(
        out.reshape(B, nbins, 2, T_out).transpose(0, 2, 1, 3)
    )


# revision 28
# speedup vs baseline: 1.0244x; 1.0244x over previous
"""CQT (constant-Q transform) kernel for Trainium2, 8 NeuronCores.

Math: out[b, c, t] = sum_l W[c, l] * x_pad[b, t*HOP + l]   (strided conv,
HOP=512, L=11339 taps, C=168 channels = 84 bins x re/im), then reshaped to
(B, 2, n_bins, T_out).

Strategy:
  - Data-parallel: shard B=32 across 8 cores (4 batches/core), weights
    replicated.
  - The conv is decomposed into 128-tap blocks: block p covers taps
    [128p, 128p+128).  The moving operand for block p=(4j+k) at output
    tile [t0, t0+nt) is a contiguous column slice of a host-pre-transposed
    view of x:  xt[r, k, u] = x_pad[512u + 128k + r].
  - CQT kernels are ragged (bin k has ~11339*2^(-k/12) taps, centered), so
    most blocks touch only a few low-bin channels.  A plain matmul costs
    ~N streaming cycles regardless of how few of the 128 PE columns hold
    weights, so the dense-block formulation wastes most of the array.
  - Column tiling: channels are split into groups of 32 (16 bins).  Each
    (block, group) quantum is a K=128, M<=32, N=nt matmul placed on one of
    the four 32-column PE tile positions (tile_position=(0, 32*slot)).
    The 4 tile positions stream concurrently, quartering PE time.
    Quanta per t-tile per group: {89, 36, 15, 7, 3, 2} = 152 vs 92
    full-width matmuls for the dense-block formulation; packed on 4 slots
    the makespan is 114 passes/batch vs 276 -> ~2.4x less PE streaming.
  - Each (group, t-tile) job accumulates its blocks into its own PSUM bank
    (per-element has_written semantics: first write overwrites, later ones
    accumulate), then DVE-copies psum[32s:32s+m] -> SBUF and DMAs to out.
    Static balanced schedule: 4 slots x 114 passes per batch.
"""

import numpy as np

HOP = 512
N_CORES = 8

_prog_cache: dict = {}


def _host_prep(x, kernels):
    x = np.ascontiguousarray(np.asarray(x, dtype=np.float32))
    kernels = np.ascontiguousarray(np.asarray(kernels, dtype=np.float32))
    B, T = x.shape
    nbins, two, Lmax = kernels.shape
    assert two == 2
    C = 2 * nbins
    pad = Lmax // 2
    T_out = (T + 2 * pad - Lmax) // HOP + 1

    # ---- weights: pad taps to 128 multiple ----
    nblk = -(-Lmax // 128)
    Wp = np.zeros((C, nblk * 128), dtype=np.float32)
    Wp[:, :Lmax] = kernels.reshape(C, Lmax)
    nzb = (Wp.reshape(C, nblk, 128) != 0.0).any(axis=2)  # [C, nblk]

    # channel groups of 32 (16 bins); bins are sorted by descending filter
    # length, supports are nested, so a group's active blocks = union over
    # its channels = the blocks of its longest (first) channel.
    groups = []  # (c0, m, blocks)
    for c0 in range(0, C, 32):
        m = min(32, C - c0)
        blks = np.where(nzb[c0:c0 + m].any(axis=0))[0].tolist()
        groups.append((c0, m, blks))

    # Weight layout: per (group, block) a zero-padded [128 taps, m chans]
    # panel.  Panels are laid out in CONSUMPTION order: wave i holds panel
    # i of every group that still has blocks (all slots consume their
    # group's panels in lockstep), so a prefix of wt's columns is exactly
    # the first waves -- weight DMA chunks can be few and large.
    maxlen_b = max(len(b) for _, _, b in groups)
    wpos = [[None] * len(blks) for _, _, blks in groups]
    wave_col = []  # column where wave i starts
    tot = 0
    for i in range(maxlen_b):
        wave_col.append(tot)
        for g, (c0, m, blks) in enumerate(groups):
            if i < len(blks):
                wpos[g][i] = tot
                tot += m
    wave_col.append(tot)
    wt = np.zeros((128, tot), dtype=np.float32)
    for g, (c0, m, blks) in enumerate(groups):
        for rel, p in enumerate(blks):
            w0 = wpos[g][rel]
            wt[:, w0: w0 + m] = Wp[c0:c0 + m, 128 * p: 128 * (p + 1)].T
    import ml_dtypes
    wt = np.ascontiguousarray(wt.astype(ml_dtypes.bfloat16))

    # ---- x: pad and pre-transpose to [128, 4, U] per batch ----
    j_max = (nblk - 1) // 4
    U = T_out + j_max
    xpad_len = 512 * U
    assert xpad_len >= pad + T, (xpad_len, pad + T)
    xp = np.zeros((B, xpad_len), dtype=np.float32)
    xp[:, pad:pad + T] = x
    # xt[b, r, k*U + u] = xp[b, 512u + 128k + r]
    import ml_dtypes
    xt = np.ascontiguousarray(
        xp.reshape(B, U, 4, 128).transpose(0, 3, 2, 1).reshape(B, 128, 4 * U)
        .astype(ml_dtypes.bfloat16)
    )
    return xt, wt, groups, (wpos, wave_col), C, U, T_out, nbins


def _build_schedule(groups, T_out, b_per):
    """Static balanced 4-slot schedule over the WHOLE core (no per-batch
    barriers).  Jobs are (batch b, group g, t-tile tt); job = len(blocks)
    passes.  Per batch the slot loads are 114/114/114/114; slots flow
    straight from one batch into the next, so PSUM/eviction reuse never
    synchronizes across slots."""
    nts = []
    t0 = 0
    while t0 < T_out:
        nts.append((t0, min(512, T_out - t0)))
        t0 += 512
    assert len(nts) == 3 and len(groups) == 6
    SLOT_JOBS = [
        [(0, 0), (2, 2), (3, 2), (4, 2)],
        [(1, 0), (1, 1), (1, 2), (5, 0), (5, 1), (5, 2)],
        [(2, 0), (3, 0), (4, 0), (0, 1)],
        [(2, 1), (3, 1), (4, 1), (0, 2)],
    ]
    # coverage check
    seen = set()
    for sj in SLOT_JOBS:
        for g, tt in sj:
            assert (g, tt) not in seen
            seen.add((g, tt))
    assert seen == {(g, tt) for g in range(6) for tt in range(3)}

    # flatten to per-slot quanta across all batches
    slot_q = []
    for sj in SLOT_JOBS:
        qs = []
        for b in range(b_per):
            for g, tt in sj:
                blks = groups[g][2]
                for rel, p in enumerate(blks):
                    qs.append(
                        (b, g, tt, p, rel, rel == 0, rel == len(blks) - 1)
                    )
        slot_q.append(qs)
    return nts, slot_q


def _build_program(b_per, C, U, T_out, groups, wmeta):
    import concourse.mybir as mybir
    import concourse.tile as tile
    from concourse import bacc

    f32 = mybir.dt.float32
    bf16 = mybir.dt.bfloat16
    wpos, wave_col = wmeta
    wtot = wave_col[-1]
    nts, slot_q = _build_schedule(groups, T_out, b_per)
    maxlen = max(len(q) for q in slot_q)

    nc = bacc.Bacc(
        "TRN2",
        target_bir_lowering=False,
        debug=False,
        enable_asserts=True,
        num_devices=N_CORES,
    )
    xt_d = nc.dram_tensor("xt", [b_per, 128, 4 * U], bf16, kind="ExternalInput").ap()
    wt_d = nc.dram_tensor("wt", [128, wtot], bf16, kind="ExternalInput").ap()
    out_d = nc.dram_tensor("out", [b_per, C, T_out], f32, kind="ExternalOutput").ap()

    # weight DMA chunk boundaries: waves 0-23, 24-47, 48+ (large
    # contiguous transfers in consumption order)
    nwave = len(wave_col) - 1
    w_cuts = [0] + [wave_col[min(w, nwave)] for w in (24, 48)] + [wtot]
    w_cuts = sorted(set(w_cuts))

    # x DMA chunk boundaries for batch 0 (u-ranges per t-tile window)
    j_max = (max(groups[0][2])) // 4
    x_stops = []
    for (t0_, nt_) in nts:
        x_stops.append(min(t0_ + nt_ + j_max + 1, U))
    x_stops[-1] = U

    with tile.TileContext(nc) as tc:
        with (
            tc.tile_pool(name="wpool", bufs=1) as wpool,
            tc.tile_pool(name="xpool", bufs=4) as xpool,
            tc.tile_pool(name="evpool", bufs=6) as evpool,
            tc.tile_pool(name="pspool", bufs=8, space="PSUM") as pspool,
        ):
            wsb = wpool.tile([128, wtot], bf16)
            xbs = [
                xpool.tile([128, 4 * U], bf16, tag="xb", name=f"xb{b}")
                for b in range(b_per)
            ]

            # PE warmup: the HAM clock gate keeps the PE at 1.2 GHz until
            # it has been busy ~3.4us.  Burn that in on scratch data while
            # the first input DMAs are in flight so the real matmuls run
            # at 2.4 GHz from pass 0.
            wu_rhs = wpool.tile([128, 512], bf16, name="wu_rhs")
            wu_ps = pspool.tile([128, 512], f32, tag="ps", name="wu_ps")
            nc.vector.memset(wu_rhs[:], 0.0)
            for _ in range(28):
                nc.tensor.matmul(
                    wu_ps[0:32, :],
                    lhsT=wu_rhs[:, 0:32],
                    rhs=wu_rhs[:],
                    start=True,
                    stop=True,
                    tile_position=(0, 0),
                )

            src0 = xt_d[0].rearrange("r (k u) -> r k u", k=4)
            dst0 = xbs[0].rearrange("r (k u) -> r k u", k=4)
            u2 = x_stops[1]
            # scalar queue: first weight wave chunk, x k-planes 2-3, then
            # the remaining weight chunks (all large transfers)
            nc.scalar.dma_start(
                out=wsb[:, w_cuts[0]:w_cuts[1]], in_=wt_d[:, w_cuts[0]:w_cuts[1]]
            )
            nc.scalar.dma_start(out=dst0[:, 2:4, 0:u2], in_=src0[:, 2:4, 0:u2])
            for a0, a1 in zip(w_cuts[1:-1], w_cuts[2:]):
                nc.scalar.dma_start(out=wsb[:, a0:a1], in_=wt_d[:, a0:a1])
            # sync queue: x k-planes 0-1, the short tail window, then
            # whole-batch prefetches
            nc.sync.dma_start(out=dst0[:, 0:2, 0:u2], in_=src0[:, 0:2, 0:u2])
            nc.sync.dma_start(out=dst0[:, :, u2:U], in_=src0[:, :, u2:U])
            for b in range(1, b_per):
                nc.sync.dma_start(out=xbs[b][:], in_=xt_d[b])

            # one global pass loop — slots flow across batch boundaries;
            # PSUM banks and eviction buffers rotate through shared pools
            # so reuse distance is ~8 jobs (many microseconds of slack).
            cur_ps = [None] * 4
            for i in range(maxlen):
                for s in range(4):
                    if i >= len(slot_q[s]):
                        continue
                    b, g, tt, p, rel, first, last = slot_q[s][i]
                    c0, m, blks = groups[g]
                    t0, nt = nts[tt]
                    xb = xbs[b]
                    if first:
                        cur_ps[s] = pspool.tile(
                            [128, 512], f32, tag="ps", name=f"ps{s}_{b}_{g}_{tt}"
                        )
                    ps = cur_ps[s]
                    j, k = divmod(p, 4)
                    wc = wpos[g][rel]
                    nc.tensor.matmul(
                        ps[32 * s: 32 * s + m, :nt],
                        lhsT=wsb[:, wc: wc + m],
                        rhs=xb[:, k * U + t0 + j: k * U + t0 + j + nt],
                        start=first,
                        stop=last,
                        tile_position=(0, 32 * s),
                    )
                    if last:
                        ev = evpool.tile(
                            [128, 512], f32, tag="ev", name=f"ev{s}_{b}_{g}_{tt}"
                        )
                        if s < 2:
                            nc.vector.tensor_copy(
                                ev[32 * s: 32 * s + m, :nt],
                                ps[32 * s: 32 * s + m, :nt],
                            )
                            nc.sync.dma_start(
                                out=out_d[b, c0:c0 + m, t0:t0 + nt],
                                in_=ev[32 * s: 32 * s + m, :nt],
                            )
                        else:
                            nc.scalar.copy(
                                ev[32 * s: 32 * s + m, :nt],
                                ps[32 * s: 32 * s + m, :nt],
                            )
                            nc.scalar.dma_start(
                                out=out_d[b, c0:c0 + m, t0:t0 + nt],
                                in_=ev[32 * s: 32 * s + m, :nt],
                            )
    nc.compile()
    return nc


def _ensure_trace_shims():
    """If run_bass_kernel_spmd is invoked with tracing enabled (e.g. via
    BASS_TRACE=1) it imports antenv.axon_hooks and uploads artifacts to a
    bucket; neither exists in a bare container.  Register a working NTFF
    hook (ctypes into the axon .so) and a no-op uploader so the trace path
    degrades gracefully instead of crashing."""
    import sys

    try:
        import antenv.axon_hooks  # noqa: F401
    except ImportError:
        import contextlib
        import ctypes
        import types

        hook = None
        try:
            lib = ctypes.CDLL("/opt/axon/libaxon_pjrt.so")
            if hasattr(lib, "axon_start_nrt_profile"):
                lib.axon_start_nrt_profile.argtypes = [
                    ctypes.POINTER(ctypes.c_int64),
                    ctypes.c_size_t,
                ]
                lib.axon_start_nrt_profile.restype = ctypes.c_int64
                lib.axon_stop_nrt_profile.argtypes = [ctypes.c_char_p]
                lib.axon_stop_nrt_profile.restype = ctypes.c_int64

                @contextlib.contextmanager
                def _hook(output_dir, device_ids):
                    import jax

                    jax.devices()
                    if device_ids:
                        ids = (ctypes.c_int64 * len(device_ids))(*device_ids)
                        rc = lib.axon_start_nrt_profile(ids, len(device_ids))
                    else:
                        rc = lib.axon_start_nrt_profile(None, 0)
                    if rc != 0:
                        raise RuntimeError(f"axon_start_nrt_profile rc={rc}")
                    try:
                        yield
                    finally:
                        lib.axon_stop_nrt_profile(str(output_dir).encode())

                hook = _hook
        except OSError:
            pass
        mod = types.ModuleType("antenv.axon_hooks")
        mod.get_axon_ntff_profile_hook = lambda: hook
        mod.set_axon_ntff_profile_hook = lambda h: None
        sys.modules["antenv.axon_hooks"] = mod

    try:
        import concourse.bass_utils as _bu

        _orig_upload = _bu.upload_artifacts

        def _safe_upload(tmpdir):
            try:
                return _orig_upload(tmpdir)
            except Exception:
                return "local://unavailable"

        if not getattr(_bu, "_safe_upload_installed", False):
            _bu.upload_artifacts = _safe_upload
            _bu._safe_upload_installed = True
    except Exception:
        pass


def kernel(x, kernels):
    _ensure_trace_shims()
    from concourse.bass_utils import run_bass_kernel_spmd

    xt, wt, groups, wmeta, C, U, T_out, nbins = _host_prep(x, kernels)
    B = xt.shape[0]
    assert B % N_CORES == 0
    b_per = B // N_CORES

    key = (b_per, C, U, T_out, tuple((c0, m, tuple(b)) for c0, m, b in groups))
    if key not in _prog_cache:
        _prog_cache[key] = _build_program(b_per, C, U, T_out, groups, wmeta)
    nc = _prog_cache[key]

    in_maps = [
        {"xt": xt[c * b_per:(c + 1) * b_per], "wt": wt} for c in range(N_CORES)
    ]
    res = run_bass_kernel_spmd(nc, in_maps, list(range(N_CORES)))
    parts = [res.results[c]["out"] for c in range(N_CORES)]
    out = np.concatenate(parts, axis=0)  # (B, C, T_out)
    return np.ascontiguousarray(
        out.reshape(B, nbins, 2, T_out).transpose(0, 2, 1, 3)
    )


# revision 31
# speedup vs baseline: 1.1220x; 1.0953x over previous
"""CQT (constant-Q transform) kernel for Trainium2, 8 NeuronCores.

Math: out[b, c, t] = sum_l W[c, l] * x_pad[b, t*HOP + l]   (strided conv,
HOP=512, L=11339 taps, C=168 channels = 84 bins x re/im), then reshaped to
(B, 2, n_bins, T_out).

Strategy:
  - Data-parallel: shard B=32 across 8 cores (4 batches/core), weights
    replicated.
  - The conv is decomposed into 128-tap blocks: block p covers taps
    [128p, 128p+128).  The moving operand for block p=(4j+k) at output
    tile [t0, t0+nt) is a contiguous column slice of a host-pre-transposed
    view of x:  xt[r, k, u] = x_pad[512u + 128k + r].
  - CQT kernels are ragged (bin k has ~11339*2^(-k/12) taps, centered), so
    most blocks touch only a few low-bin channels.  A plain matmul costs
    ~N streaming cycles regardless of how few of the 128 PE columns hold
    weights, so the dense-block formulation wastes most of the array.
  - Column tiling: channels are split into groups of 32 (16 bins).  Each
    (block, group) quantum is a K=128, M<=32, N=nt matmul placed on one of
    the four 32-column PE tile positions (tile_position=(0, 32*slot)).
    The 4 tile positions stream concurrently, quartering PE time.
    Quanta per t-tile per group: {89, 36, 15, 7, 3, 2} = 152 vs 92
    full-width matmuls for the dense-block formulation; packed on 4 slots
    the makespan is 114 passes/batch vs 276 -> ~2.4x less PE streaming.
  - Each (group, t-tile) job accumulates its blocks into its own PSUM bank
    (per-element has_written semantics: first write overwrites, later ones
    accumulate), then DVE-copies psum[32s:32s+m] -> SBUF and DMAs to out.
    Static balanced schedule: 4 slots x 114 passes per batch.
"""

import numpy as np

HOP = 512
N_CORES = 8

_prog_cache: dict = {}


def _host_prep(x, kernels):
    x = np.ascontiguousarray(np.asarray(x, dtype=np.float32))
    kernels = np.ascontiguousarray(np.asarray(kernels, dtype=np.float32))
    B, T = x.shape
    nbins, two, Lmax = kernels.shape
    assert two == 2
    C = 2 * nbins
    pad = Lmax // 2
    T_out = (T + 2 * pad - Lmax) // HOP + 1

    # ---- weights: pad taps to 128 multiple ----
    nblk = -(-Lmax // 128)
    Wp = np.zeros((C, nblk * 128), dtype=np.float32)
    Wp[:, :Lmax] = kernels.reshape(C, Lmax)
    nzb = (Wp.reshape(C, nblk, 128) != 0.0).any(axis=2)  # [C, nblk]

    # channel groups of 32 (16 bins); bins are sorted by descending filter
    # length, supports are nested, so a group's active blocks = union over
    # its channels = the blocks of its longest (first) channel.
    groups = []  # (c0, m, blocks)
    for c0 in range(0, C, 32):
        m = min(32, C - c0)
        blks = np.where(nzb[c0:c0 + m].any(axis=0))[0].tolist()
        groups.append((c0, m, blks))

    # Weight layout: per (group, block) a zero-padded [128 taps, m chans]
    # panel.  Panels are laid out in CONSUMPTION order: wave i holds panel
    # i of every group that still has blocks (all slots consume their
    # group's panels in lockstep), so a prefix of wt's columns is exactly
    # the first waves -- weight DMA chunks can be few and large.
    maxlen_b = max(len(b) for _, _, b in groups)
    wpos = [[None] * len(blks) for _, _, blks in groups]
    wave_col = []  # column where wave i starts
    tot = 0
    for i in range(maxlen_b):
        wave_col.append(tot)
        for g, (c0, m, blks) in enumerate(groups):
            if i < len(blks):
                wpos[g][i] = tot
                tot += m
    wave_col.append(tot)
    wt = np.zeros((128, tot), dtype=np.float32)
    for g, (c0, m, blks) in enumerate(groups):
        for rel, p in enumerate(blks):
            w0 = wpos[g][rel]
            wt[:, w0: w0 + m] = Wp[c0:c0 + m, 128 * p: 128 * (p + 1)].T
    import ml_dtypes
    wt = np.ascontiguousarray(wt.astype(ml_dtypes.bfloat16))

    # ---- x: pad and pre-transpose to [128, 4, U] per batch ----
    j_max = (nblk - 1) // 4
    U = T_out + j_max
    xpad_len = 512 * U
    assert xpad_len >= pad + T, (xpad_len, pad + T)
    xp = np.zeros((B, xpad_len), dtype=np.float32)
    xp[:, pad:pad + T] = x
    # xt[b, r, k*U + u] = xp[b, 512u + 128k + r]
    import ml_dtypes
    xt = np.ascontiguousarray(
        xp.reshape(B, U, 4, 128).transpose(0, 3, 2, 1).reshape(B, 128, 4 * U)
        .astype(ml_dtypes.bfloat16)
    )
    return xt, wt, groups, (wpos, wave_col), C, U, T_out, nbins


def _build_schedule(groups, T_out, b_per):
    """Static balanced 4-slot schedule over the WHOLE core (no per-batch
    barriers).  Jobs are (batch b, group g, t-tile tt); job = len(blocks)
    passes.  Per batch the slot loads are 114/114/114/114; slots flow
    straight from one batch into the next, so PSUM/eviction reuse never
    synchronizes across slots."""
    nts = []
    t0 = 0
    while t0 < T_out:
        nts.append((t0, min(512, T_out - t0)))
        t0 += 512
    assert len(nts) == 3 and len(groups) == 6
    # Slot loads balanced by STREAMING CYCLES (sum of quantum widths), not
    # pass count: the last t-tile is only 268 wide, and the in-order PE
    # queue couples all four tile positions to the slowest slot.  Cycle
    # loads per batch: 49.0k / 49.6k / 50.0k / 47.7k (vs 58.4k max for the
    # pass-balanced schedule).  Job order within a slot starts batch 0
    # from the lowest t-tiles it owns.
    SLOT_JOBS = [
        [(0, 0), (3, 2), (4, 1)],
        [(0, 1), (2, 2)],
        [(1, 0), (2, 0), (0, 2)],
        [(3, 0), (4, 0), (5, 0), (1, 1), (2, 1), (3, 1), (5, 1),
         (1, 2), (4, 2), (5, 2)],
    ]
    # coverage check
    seen = set()
    for sj in SLOT_JOBS:
        for g, tt in sj:
            assert (g, tt) not in seen
            seen.add((g, tt))
    assert seen == {(g, tt) for g in range(6) for tt in range(3)}

    # flatten to per-slot quanta across all batches
    slot_q = []
    for sj in SLOT_JOBS:
        qs = []
        for b in range(b_per):
            for g, tt in sj:
                blks = groups[g][2]
                for rel, p in enumerate(blks):
                    qs.append(
                        (b, g, tt, p, rel, rel == 0, rel == len(blks) - 1)
                    )
        slot_q.append(qs)

    # merge to one emission list in VIRTUAL-TIME order: always emit for
    # the slot with the least streaming cycles issued so far, so the PE
    # queue (strict in-order issue) interleaves quanta in roughly the
    # order the tile positions actually free up.
    clocks = [0.0] * 4
    idx = [0] * 4
    emit = []
    while any(idx[s] < len(slot_q[s]) for s in range(4)):
        s = min(
            (s for s in range(4) if idx[s] < len(slot_q[s])),
            key=lambda s: (clocks[s], s),
        )
        q = slot_q[s][idx[s]]
        idx[s] += 1
        emit.append((s,) + q)
        clocks[s] += nts[q[2]][1]
    return nts, emit


def _build_program(b_per, C, U, T_out, groups, wmeta):
    import concourse.mybir as mybir
    import concourse.tile as tile
    from concourse import bacc

    f32 = mybir.dt.float32
    bf16 = mybir.dt.bfloat16
    wpos, wave_col = wmeta
    wtot = wave_col[-1]
    nts, emit = _build_schedule(groups, T_out, b_per)

    nc = bacc.Bacc(
        "TRN2",
        target_bir_lowering=False,
        debug=False,
        enable_asserts=True,
        num_devices=N_CORES,
    )
    xt_d = nc.dram_tensor("xt", [b_per, 128, 4 * U], bf16, kind="ExternalInput").ap()
    wt_d = nc.dram_tensor("wt", [128, wtot], bf16, kind="ExternalInput").ap()
    out_d = nc.dram_tensor("out", [b_per, C, T_out], f32, kind="ExternalOutput").ap()

    # weight DMA chunk boundaries: waves 0-23, 24-47, 48+ (large
    # contiguous transfers in consumption order)
    nwave = len(wave_col) - 1
    w_cuts = [0] + [wave_col[min(w, nwave)] for w in (24, 48)] + [wtot]
    w_cuts = sorted(set(w_cuts))

    # x DMA chunk boundaries for batch 0 (u-ranges per t-tile window)
    j_max = (max(groups[0][2])) // 4
    x_stops = []
    for (t0_, nt_) in nts:
        x_stops.append(min(t0_ + nt_ + j_max + 1, U))
    x_stops[-1] = U

    with tile.TileContext(nc) as tc:
        with (
            tc.tile_pool(name="wpool", bufs=1) as wpool,
            tc.tile_pool(name="xpool", bufs=4) as xpool,
            tc.tile_pool(name="evpool", bufs=6) as evpool,
            tc.tile_pool(name="pspool", bufs=8, space="PSUM") as pspool,
        ):
            wsb = wpool.tile([128, wtot], bf16)
            xbs = [
                xpool.tile([128, 4 * U], bf16, tag="xb", name=f"xb{b}")
                for b in range(b_per)
            ]

            # PE warmup: the HAM clock gate keeps the PE at 1.2 GHz until
            # it has been busy ~3.4us.  Burn that in on scratch data while
            # the first input DMAs are in flight so the real matmuls run
            # at 2.4 GHz from pass 0.
            wu_rhs = wpool.tile([128, 512], bf16, name="wu_rhs")
            wu_ps = pspool.tile([128, 512], f32, tag="ps", name="wu_ps")
            nc.vector.memset(wu_rhs[:], 0.0)
            for _ in range(28):
                nc.tensor.matmul(
                    wu_ps[0:32, :],
                    lhsT=wu_rhs[:, 0:32],
                    rhs=wu_rhs[:],
                    start=True,
                    stop=True,
                    tile_position=(0, 0),
                )

            src0 = xt_d[0].rearrange("r (k u) -> r k u", k=4)
            dst0 = xbs[0].rearrange("r (k u) -> r k u", k=4)
            u2 = x_stops[1]
            # scalar queue: first weight wave chunk, x k-planes 2-3, then
            # the remaining weight chunks (all large transfers)
            nc.scalar.dma_start(
                out=wsb[:, w_cuts[0]:w_cuts[1]], in_=wt_d[:, w_cuts[0]:w_cuts[1]]
            )
            nc.scalar.dma_start(out=dst0[:, 2:4, 0:u2], in_=src0[:, 2:4, 0:u2])
            for a0, a1 in zip(w_cuts[1:-1], w_cuts[2:]):
                nc.scalar.dma_start(out=wsb[:, a0:a1], in_=wt_d[:, a0:a1])
            # sync queue: x k-planes 0-1, the short tail window, then
            # whole-batch prefetches
            nc.sync.dma_start(out=dst0[:, 0:2, 0:u2], in_=src0[:, 0:2, 0:u2])
            nc.sync.dma_start(out=dst0[:, :, u2:U], in_=src0[:, :, u2:U])
            for b in range(1, b_per):
                nc.sync.dma_start(out=xbs[b][:], in_=xt_d[b])

            # one global emission loop in virtual-time order — slots flow
            # across batch boundaries; PSUM banks and eviction buffers
            # rotate through shared pools so reuse distance is ~8 jobs.
            cur_ps = [None] * 4
            for (s, b, g, tt, p, rel, first, last) in emit:
                c0, m, blks = groups[g]
                t0, nt = nts[tt]
                xb = xbs[b]
                if first:
                    cur_ps[s] = pspool.tile(
                        [128, 512], f32, tag="ps", name=f"ps{s}_{b}_{g}_{tt}"
                    )
                ps = cur_ps[s]
                j, k = divmod(p, 4)
                wc = wpos[g][rel]
                nc.tensor.matmul(
                    ps[32 * s: 32 * s + m, :nt],
                    lhsT=wsb[:, wc: wc + m],
                    rhs=xb[:, k * U + t0 + j: k * U + t0 + j + nt],
                    start=first,
                    stop=last,
                    tile_position=(0, 32 * s),
                )
                if last:
                    ev = evpool.tile(
                        [128, 512], f32, tag="ev", name=f"ev{s}_{b}_{g}_{tt}"
                    )
                    if s < 2:
                        nc.vector.tensor_copy(
                            ev[32 * s: 32 * s + m, :nt],
                            ps[32 * s: 32 * s + m, :nt],
                        )
                        nc.sync.dma_start(
                            out=out_d[b, c0:c0 + m, t0:t0 + nt],
                            in_=ev[32 * s: 32 * s + m, :nt],
                        )
                    else:
                        nc.scalar.copy(
                            ev[32 * s: 32 * s + m, :nt],
                            ps[32 * s: 32 * s + m, :nt],
                        )
                        nc.scalar.dma_start(
                            out=out_d[b, c0:c0 + m, t0:t0 + nt],
                            in_=ev[32 * s: 32 * s + m, :nt],
                        )
    nc.compile()
    return nc


def _ensure_trace_shims():
    """If run_bass_kernel_spmd is invoked with tracing enabled (e.g. via
    BASS_TRACE=1) it imports antenv.axon_hooks and uploads artifacts to a
    bucket; neither exists in a bare container.  Register a working NTFF
    hook (ctypes into the axon .so) and a no-op uploader so the trace path
    degrades gracefully instead of crashing."""
    import sys

    try:
        import antenv.axon_hooks  # noqa: F401
    except ImportError:
        import contextlib
        import ctypes
        import types

        hook = None
        try:
            lib = ctypes.CDLL("/opt/axon/libaxon_pjrt.so")
            if hasattr(lib, "axon_start_nrt_profile"):
                lib.axon_start_nrt_profile.argtypes = [
                    ctypes.POINTER(ctypes.c_int64),
                    ctypes.c_size_t,
                ]
                lib.axon_start_nrt_profile.restype = ctypes.c_int64
                lib.axon_stop_nrt_profile.argtypes = [ctypes.c_char_p]
                lib.axon_stop_nrt_profile.restype = ctypes.c_int64

                @contextlib.contextmanager
                def _hook(output_dir, device_ids):
                    import jax

                    jax.devices()
                    if device_ids:
                        ids = (ctypes.c_int64 * len(device_ids))(*device_ids)
                        rc = lib.axon_start_nrt_profile(ids, len(device_ids))
                    else:
                        rc = lib.axon_start_nrt_profile(None, 0)
                    if rc != 0:
                        raise RuntimeError(f"axon_start_nrt_profile rc={rc}")
                    try:
                        yield
                    finally:
                        lib.axon_stop_nrt_profile(str(output_dir).encode())

                hook = _hook
        except OSError:
            pass
        mod = types.ModuleType("antenv.axon_hooks")
        mod.get_axon_ntff_profile_hook = lambda: hook
        mod.set_axon_ntff_profile_hook = lambda h: None
        sys.modules["antenv.axon_hooks"] = mod

    try:
        import concourse.bass_utils as _bu

        _orig_upload = _bu.upload_artifacts

        def _safe_upload(tmpdir):
            try:
                return _orig_upload(tmpdir)
            except Exception:
                return "local://unavailable"

        if not getattr(_bu, "_safe_upload_installed", False):
            _bu.upload_artifacts = _safe_upload
            _bu._safe_upload_installed = True
    except Exception:
        pass


def kernel(x, kernels):
    _ensure_trace_shims()
    from concourse.bass_utils import run_bass_kernel_spmd

    xt, wt, groups, wmeta, C, U, T_out, nbins = _host_prep(x, kernels)
    B = xt.shape[0]
    assert B % N_CORES == 0
    b_per = B // N_CORES

    key = (b_per, C, U, T_out, tuple((c0, m, tuple(b)) for c0, m, b in groups))
    if key not in _prog_cache:
        _prog_cache[key] = _build_program(b_per, C, U, T_out, groups, wmeta)
    nc = _prog_cache[key]

    in_maps = [
        {"xt": xt[c * b_per:(c + 1) * b_per], "wt": wt} for c in range(N_CORES)
    ]
    res = run_bass_kernel_spmd(nc, in_maps, list(range(N_CORES)))
    parts = [res.results[c]["out"] for c in range(N_CORES)]
    out = np.concatenate(parts, axis=0)  # (B, C, T_out)
    return np.ascontiguousarray(
        out.reshape(B, nbins, 2, T_out).transpose(0, 2, 1, 3)
    )


# revision 32
# speedup vs baseline: 1.1325x; 1.0093x over previous
"""CQT (constant-Q transform) kernel for Trainium2, 8 NeuronCores.

Math: out[b, c, t] = sum_l W[c, l] * x_pad[b, t*HOP + l]   (strided conv,
HOP=512, L=11339 taps, C=168 channels = 84 bins x re/im), then reshaped to
(B, 2, n_bins, T_out).

Strategy:
  - Data-parallel: shard B=32 across 8 cores (4 batches/core), weights
    replicated.
  - The conv is decomposed into 128-tap blocks: block p covers taps
    [128p, 128p+128).  The moving operand for block p=(4j+k) at output
    tile [t0, t0+nt) is a contiguous column slice of a host-pre-transposed
    view of x:  xt[r, k, u] = x_pad[512u + 128k + r].
  - CQT kernels are ragged (bin k has ~11339*2^(-k/12) taps, centered), so
    most blocks touch only a few low-bin channels.  A plain matmul costs
    ~N streaming cycles regardless of how few of the 128 PE columns hold
    weights, so the dense-block formulation wastes most of the array.
  - Column tiling: channels are split into groups of 32 (16 bins).  Each
    (block, group) quantum is a K=128, M<=32, N=nt bf16 matmul placed on
    one of the four 32-column PE tile positions (tile_position=(0, 32*s)).
    The 4 tile positions stream concurrently, quartering PE time.
    Quanta per t-tile per group: {89, 36, 15, 7, 3, 2} = 152 vs 92
    full-width matmuls for the dense-block formulation.
  - Jobs (batch, group, t-tile) are assigned to slots balanced by
    STREAMING CYCLES (~49k/slot/batch) and emitted in virtual-time order
    with no batch barriers; PSUM banks (8) and eviction buffers rotate
    through shared pools so reuse never stalls the in-order PE queue.
  - Each job accumulates its blocks into one PSUM bank (per-element
    has_written: first write overwrites, later ones accumulate), then is
    copied psum[32s:32s+m] -> SBUF (vector for slots 0-1, scalar for 2-3)
    and DMA'd out (sync / scalar HW queues).
  - Warmup matmuls on scratch data run during the input DMAs so the HAM
    clock gate is already at 2.4 GHz when the real stream starts; inputs
    arrive as a few large wave-ordered transfers on two HW DMA queues.
"""

import numpy as np

HOP = 512
N_CORES = 8

_prog_cache: dict = {}


def _host_prep(x, kernels):
    x = np.ascontiguousarray(np.asarray(x, dtype=np.float32))
    kernels = np.ascontiguousarray(np.asarray(kernels, dtype=np.float32))
    B, T = x.shape
    nbins, two, Lmax = kernels.shape
    assert two == 2
    C = 2 * nbins
    pad = Lmax // 2
    T_out = (T + 2 * pad - Lmax) // HOP + 1

    # ---- weights: pad taps to 128 multiple ----
    nblk = -(-Lmax // 128)
    Wp = np.zeros((C, nblk * 128), dtype=np.float32)
    Wp[:, :Lmax] = kernels.reshape(C, Lmax)
    nzb = (Wp.reshape(C, nblk, 128) != 0.0).any(axis=2)  # [C, nblk]

    # channel groups of 32 (16 bins); bins are sorted by descending filter
    # length, supports are nested, so a group's active blocks = union over
    # its channels = the blocks of its longest (first) channel.
    groups = []  # (c0, m, blocks)
    for c0 in range(0, C, 32):
        m = min(32, C - c0)
        blks = np.where(nzb[c0:c0 + m].any(axis=0))[0].tolist()
        groups.append((c0, m, blks))

    # Weight layout: per (group, block) a zero-padded [128 taps, m chans]
    # panel.  Panels are laid out in CONSUMPTION order: wave i holds panel
    # i of every group that still has blocks (all slots consume their
    # group's panels in lockstep), so a prefix of wt's columns is exactly
    # the first waves -- weight DMA chunks can be few and large.
    maxlen_b = max(len(b) for _, _, b in groups)
    wpos = [[None] * len(blks) for _, _, blks in groups]
    wave_col = []  # column where wave i starts
    tot = 0
    for i in range(maxlen_b):
        wave_col.append(tot)
        for g, (c0, m, blks) in enumerate(groups):
            if i < len(blks):
                wpos[g][i] = tot
                tot += m
    wave_col.append(tot)
    wt = np.zeros((128, tot), dtype=np.float32)
    for g, (c0, m, blks) in enumerate(groups):
        for rel, p in enumerate(blks):
            w0 = wpos[g][rel]
            wt[:, w0: w0 + m] = Wp[c0:c0 + m, 128 * p: 128 * (p + 1)].T
    import ml_dtypes
    wt = np.ascontiguousarray(wt.astype(ml_dtypes.bfloat16))

    # ---- x: pad and pre-transpose to [128, 4, U] per batch ----
    j_max = (nblk - 1) // 4
    U = T_out + j_max
    xpad_len = 512 * U
    assert xpad_len >= pad + T, (xpad_len, pad + T)
    xp = np.zeros((B, xpad_len), dtype=np.float32)
    xp[:, pad:pad + T] = x
    # xt[b, r, k*U + u] = xp[b, 512u + 128k + r]
    import ml_dtypes
    xt = np.ascontiguousarray(
        xp.reshape(B, U, 4, 128).transpose(0, 3, 2, 1).reshape(B, 128, 4 * U)
        .astype(ml_dtypes.bfloat16)
    )
    return xt, wt, groups, (wpos, wave_col), C, U, T_out, nbins


def _build_schedule(groups, T_out, b_per):
    """Static balanced 4-slot schedule over the WHOLE core (no per-batch
    barriers).  Jobs are (batch b, group g, t-tile tt); job = len(blocks)
    passes.  Per batch the slot loads are 114/114/114/114; slots flow
    straight from one batch into the next, so PSUM/eviction reuse never
    synchronizes across slots."""
    nts = []
    t0 = 0
    while t0 < T_out:
        nts.append((t0, min(512, T_out - t0)))
        t0 += 512
    assert len(nts) == 3 and len(groups) == 6
    # Slot loads balanced by STREAMING CYCLES (sum of quantum widths), not
    # pass count: the last t-tile is only 268 wide, and the in-order PE
    # queue couples all four tile positions to the slowest slot.  Cycle
    # loads per batch: 49.0k / 49.6k / 50.0k / 47.7k (vs 58.4k max for the
    # pass-balanced schedule).  Job order within a slot starts batch 0
    # from the lowest t-tiles it owns.
    SLOT_JOBS = [
        [(0, 0), (3, 2), (4, 1)],
        [(0, 1), (2, 2)],
        [(1, 0), (2, 0), (0, 2)],
        [(3, 0), (4, 0), (5, 0), (1, 1), (2, 1), (3, 1), (5, 1),
         (1, 2), (4, 2), (5, 2)],
    ]
    # coverage check
    seen = set()
    for sj in SLOT_JOBS:
        for g, tt in sj:
            assert (g, tt) not in seen
            seen.add((g, tt))
    assert seen == {(g, tt) for g in range(6) for tt in range(3)}

    # flatten to per-slot quanta across all batches
    slot_q = []
    for sj in SLOT_JOBS:
        qs = []
        for b in range(b_per):
            for g, tt in sj:
                blks = groups[g][2]
                for rel, p in enumerate(blks):
                    qs.append(
                        (b, g, tt, p, rel, rel == 0, rel == len(blks) - 1)
                    )
        slot_q.append(qs)

    # merge to one emission list in VIRTUAL-TIME order: always emit for
    # the slot with the least streaming cycles issued so far, so the PE
    # queue (strict in-order issue) interleaves quanta in roughly the
    # order the tile positions actually free up.
    clocks = [0.0] * 4
    idx = [0] * 4
    emit = []
    while any(idx[s] < len(slot_q[s]) for s in range(4)):
        s = min(
            (s for s in range(4) if idx[s] < len(slot_q[s])),
            key=lambda s: (clocks[s], s),
        )
        q = slot_q[s][idx[s]]
        idx[s] += 1
        emit.append((s,) + q)
        clocks[s] += nts[q[2]][1]
    return nts, emit


def _build_program(b_per, C, U, T_out, groups, wmeta):
    import concourse.mybir as mybir
    import concourse.tile as tile
    from concourse import bacc

    f32 = mybir.dt.float32
    bf16 = mybir.dt.bfloat16
    wpos, wave_col = wmeta
    wtot = wave_col[-1]
    nts, emit = _build_schedule(groups, T_out, b_per)

    nc = bacc.Bacc(
        "TRN2",
        target_bir_lowering=False,
        debug=False,
        enable_asserts=True,
        num_devices=N_CORES,
    )
    xt_d = nc.dram_tensor("xt", [b_per, 128, 4 * U], bf16, kind="ExternalInput").ap()
    wt_d = nc.dram_tensor("wt", [128, wtot], bf16, kind="ExternalInput").ap()
    out_d = nc.dram_tensor("out", [b_per, C, T_out], f32, kind="ExternalOutput").ap()

    # weight DMA chunk boundaries: waves 0-23, 24-47, 48+ (large
    # contiguous transfers in consumption order)
    nwave = len(wave_col) - 1
    w_cuts = [0] + [wave_col[min(w, nwave)] for w in (24, 48)] + [wtot]
    w_cuts = sorted(set(w_cuts))

    # x DMA chunk boundaries for batch 0 (u-ranges per t-tile window)
    j_max = (max(groups[0][2])) // 4
    x_stops = []
    for (t0_, nt_) in nts:
        x_stops.append(min(t0_ + nt_ + j_max + 1, U))
    x_stops[-1] = U

    with tile.TileContext(nc) as tc:
        with (
            tc.tile_pool(name="wpool", bufs=1) as wpool,
            tc.tile_pool(name="xpool", bufs=4) as xpool,
            tc.tile_pool(name="evpool", bufs=6) as evpool,
            tc.tile_pool(name="pspool", bufs=8, space="PSUM") as pspool,
        ):
            wsb = wpool.tile([128, wtot], bf16)
            xbs = [
                xpool.tile([128, 4 * U], bf16, tag="xb", name=f"xb{b}")
                for b in range(b_per)
            ]

            # PE warmup: the HAM clock gate keeps the PE at 1.2 GHz until
            # it has been busy ~3.4us.  Burn that in on scratch data while
            # the first input DMAs are in flight so the real matmuls run
            # at 2.4 GHz from pass 0.
            wu_rhs = wpool.tile([128, 512], bf16, name="wu_rhs")
            wu_ps = pspool.tile([128, 512], f32, tag="ps", name="wu_ps")
            nc.vector.memset(wu_rhs[:], 0.0)
            for _ in range(28):
                nc.tensor.matmul(
                    wu_ps[0:32, :],
                    lhsT=wu_rhs[:, 0:32],
                    rhs=wu_rhs[:],
                    start=True,
                    stop=True,
                    tile_position=(0, 0),
                )

            src0 = xt_d[0].rearrange("r (k u) -> r k u", k=4)
            dst0 = xbs[0].rearrange("r (k u) -> r k u", k=4)
            u2 = x_stops[1]
            # scalar queue: first weight wave chunk, x k-planes 2-3, then
            # the remaining weight chunks (all large transfers)
            nc.scalar.dma_start(
                out=wsb[:, w_cuts[0]:w_cuts[1]], in_=wt_d[:, w_cuts[0]:w_cuts[1]]
            )
            nc.scalar.dma_start(out=dst0[:, 2:4, 0:u2], in_=src0[:, 2:4, 0:u2])
            for a0, a1 in zip(w_cuts[1:-1], w_cuts[2:]):
                nc.scalar.dma_start(out=wsb[:, a0:a1], in_=wt_d[:, a0:a1])
            # sync queue: x k-planes 0-1, the short tail window, then
            # whole-batch prefetches
            nc.sync.dma_start(out=dst0[:, 0:2, 0:u2], in_=src0[:, 0:2, 0:u2])
            nc.sync.dma_start(out=dst0[:, :, u2:U], in_=src0[:, :, u2:U])
            for b in range(1, b_per):
                nc.sync.dma_start(out=xbs[b][:], in_=xt_d[b])

            # one global emission loop in virtual-time order — slots flow
            # across batch boundaries; PSUM banks and eviction buffers
            # rotate through shared pools so reuse distance is ~8 jobs.
            cur_ps = [None] * 4
            for (s, b, g, tt, p, rel, first, last) in emit:
                c0, m, blks = groups[g]
                t0, nt = nts[tt]
                xb = xbs[b]
                if first:
                    cur_ps[s] = pspool.tile(
                        [128, 512], f32, tag="ps", name=f"ps{s}_{b}_{g}_{tt}"
                    )
                ps = cur_ps[s]
                j, k = divmod(p, 4)
                wc = wpos[g][rel]
                nc.tensor.matmul(
                    ps[32 * s: 32 * s + m, :nt],
                    lhsT=wsb[:, wc: wc + m],
                    rhs=xb[:, k * U + t0 + j: k * U + t0 + j + nt],
                    start=first,
                    stop=last,
                    tile_position=(0, 32 * s),
                )
                if last:
                    ev = evpool.tile(
                        [128, 512], f32, tag="ev", name=f"ev{s}_{b}_{g}_{tt}"
                    )
                    if s < 2:
                        nc.vector.tensor_copy(
                            ev[32 * s: 32 * s + m, :nt],
                            ps[32 * s: 32 * s + m, :nt],
                        )
                        nc.sync.dma_start(
                            out=out_d[b, c0:c0 + m, t0:t0 + nt],
                            in_=ev[32 * s: 32 * s + m, :nt],
                        )
                    else:
                        nc.scalar.copy(
                            ev[32 * s: 32 * s + m, :nt],
                            ps[32 * s: 32 * s + m, :nt],
                        )
                        nc.scalar.dma_start(
                            out=out_d[b, c0:c0 + m, t0:t0 + nt],
                            in_=ev[32 * s: 32 * s + m, :nt],
                        )
    nc.compile()
    return nc


def _ensure_trace_shims():
    """If run_bass_kernel_spmd is invoked with tracing enabled (e.g. via
    BASS_TRACE=1) it imports antenv.axon_hooks and uploads artifacts to a
    bucket; neither exists in a bare container.  Register a working NTFF
    hook (ctypes into the axon .so) and a no-op uploader so the trace path
    degrades gracefully instead of crashing."""
    import sys

    try:
        import antenv.axon_hooks  # noqa: F401
    except ImportError:
        import contextlib
        import ctypes
        import types

        hook = None
        try:
            lib = ctypes.CDLL("/opt/axon/libaxon_pjrt.so")
            if hasattr(lib, "axon_start_nrt_profile"):
                lib.axon_start_nrt_profile.argtypes = [
                    ctypes.POINTER(ctypes.c_int64),
                    ctypes.c_size_t,
                ]
                lib.axon_start_nrt_profile.restype = ctypes.c_int64
                lib.axon_stop_nrt_profile.argtypes = [ctypes.c_char_p]
                lib.axon_stop_nrt_profile.restype = ctypes.c_int64

                @contextlib.contextmanager
                def _hook(output_dir, device_ids):
                    import jax

                    jax.devices()
                    if device_ids:
                        ids = (ctypes.c_int64 * len(device_ids))(*device_ids)
                        rc = lib.axon_start_nrt_profile(ids, len(device_ids))
                    else:
                        rc = lib.axon_start_nrt_profile(None, 0)
                    if rc != 0:
                        raise RuntimeError(f"axon_start_nrt_profile rc={rc}")
                    try:
                        yield
                    finally:
                        lib.axon_stop_nrt_profile(str(output_dir).encode())

                hook = _hook
        except OSError:
            pass
        mod = types.ModuleType("antenv.axon_hooks")
        mod.get_axon_ntff_profile_hook = lambda: hook
        mod.set_axon_ntff_profile_hook = lambda h: None
        sys.modules["antenv.axon_hooks"] = mod

    try:
        import concourse.bass_utils as _bu

        _orig_upload = _bu.upload_artifacts

        def _safe_upload(tmpdir):
            try:
                return _orig_upload(tmpdir)
            except Exception:
                return "local://unavailable"

        if not getattr(_bu, "_safe_upload_installed", False):
            _bu.upload_artifacts = _safe_upload
            _bu._safe_upload_installed = True
    except Exception:
        pass


def kernel(x, kernels):
    _ensure_trace_shims()
    from concourse.bass_utils import run_bass_kernel_spmd

    xt, wt, groups, wmeta, C, U, T_out, nbins = _host_prep(x, kernels)
    B = xt.shape[0]
    assert B % N_CORES == 0
    b_per = B // N_CORES

    key = (b_per, C, U, T_out, tuple((c0, m, tuple(b)) for c0, m, b in groups))
    if key not in _prog_cache:
        _prog_cache[key] = _build_program(b_per, C, U, T_out, groups, wmeta)
    nc = _prog_cache[key]

    in_maps = [
        {"xt": xt[c * b_per:(c + 1) * b_per], "wt": wt} for c in range(N_CORES)
    ]
    res = run_bass_kernel_spmd(nc, in_maps, list(range(N_CORES)))
    parts = [res.results[c]["out"] for c in range(N_CORES)]
    out = np.concatenate(parts, axis=0)  # (B, C, T_out)
    return np.ascontiguousarray(
        out.reshape(B, nbins, 2, T_out).transpose(0, 2, 1, 3)
    )


# revision 33
# speedup vs baseline: 1.1586x; 1.0231x over previous
"""CQT (constant-Q transform) kernel for Trainium2, 8 NeuronCores.

Math: out[b, c, t] = sum_l W[c, l] * x_pad[b, t*HOP + l]   (strided conv,
HOP=512, L=11339 taps, C=168 channels = 84 bins x re/im), then reshaped to
(B, 2, n_bins, T_out).

Strategy:
  - Data-parallel: shard B=32 across 8 cores (4 batches/core), weights
    replicated.
  - The conv is decomposed into 128-tap blocks: block p covers taps
    [128p, 128p+128).  The moving operand for block p=(4j+k) at output
    tile [t0, t0+nt) is a contiguous column slice of a host-pre-transposed
    view of x:  xt[r, k, u] = x_pad[512u + 128k + r].
  - CQT kernels are ragged (bin k has ~11339*2^(-k/12) taps, centered), so
    most blocks touch only a few low-bin channels.  A plain matmul costs
    ~N streaming cycles regardless of how few of the 128 PE columns hold
    weights, so the dense-block formulation wastes most of the array.
  - Column tiling: channels are split into groups of 32 (16 bins).  Each
    (block, group) quantum is a K=128, M<=32, N=nt bf16 matmul placed on
    one of the four 32-column PE tile positions (tile_position=(0, 32*s)).
    The 4 tile positions stream concurrently, quartering PE time.
    Quanta per t-tile per group: {89, 36, 15, 7, 3, 2} = 152 vs 92
    full-width matmuls for the dense-block formulation.
  - Jobs (batch, group, t-tile) are assigned to slots balanced by
    STREAMING CYCLES (~49k/slot/batch) and emitted in virtual-time order
    with no batch barriers; PSUM banks (8) and eviction buffers rotate
    through shared pools so reuse never stalls the in-order PE queue.
  - Each job accumulates its blocks into one PSUM bank (per-element
    has_written: first write overwrites, later ones accumulate), then is
    copied psum[32s:32s+m] -> SBUF (vector for slots 0-1, scalar for 2-3)
    and DMA'd out (sync / scalar HW queues).
  - Warmup matmuls on scratch data run during the input DMAs so the HAM
    clock gate is already at 2.4 GHz when the real stream starts; inputs
    arrive as a few large wave-ordered transfers on two HW DMA queues.
"""

import numpy as np

HOP = 512
N_CORES = 8

_prog_cache: dict = {}


def _host_prep(x, kernels):
    x = np.ascontiguousarray(np.asarray(x, dtype=np.float32))
    kernels = np.ascontiguousarray(np.asarray(kernels, dtype=np.float32))
    B, T = x.shape
    nbins, two, Lmax = kernels.shape
    assert two == 2
    C = 2 * nbins
    pad = Lmax // 2
    T_out = (T + 2 * pad - Lmax) // HOP + 1

    # ---- weights: pad taps to 128 multiple ----
    nblk = -(-Lmax // 128)
    Wp = np.zeros((C, nblk * 128), dtype=np.float32)
    Wp[:, :Lmax] = kernels.reshape(C, Lmax)
    nzb = (Wp.reshape(C, nblk, 128) != 0.0).any(axis=2)  # [C, nblk]

    # channel groups of 32 (16 bins); bins are sorted by descending filter
    # length, supports are nested, so a group's active blocks = union over
    # its channels = the blocks of its longest (first) channel.
    groups = []  # (c0, m, blocks)
    for c0 in range(0, C, 32):
        m = min(32, C - c0)
        blks = np.where(nzb[c0:c0 + m].any(axis=0))[0].tolist()
        groups.append((c0, m, blks))

    # Weight layout: per (group, block) a zero-padded [128 taps, m chans]
    # panel.  Panels are laid out in CONSUMPTION order: wave i holds panel
    # i of every group that still has blocks (all slots consume their
    # group's panels in lockstep), so a prefix of wt's columns is exactly
    # the first waves -- weight DMA chunks can be few and large.
    maxlen_b = max(len(b) for _, _, b in groups)
    wpos = [[None] * len(blks) for _, _, blks in groups]
    wave_col = []  # column where wave i starts
    tot = 0
    for i in range(maxlen_b):
        wave_col.append(tot)
        for g, (c0, m, blks) in enumerate(groups):
            if i < len(blks):
                wpos[g][i] = tot
                tot += m
    wave_col.append(tot)
    wt = np.zeros((128, tot), dtype=np.float32)
    for g, (c0, m, blks) in enumerate(groups):
        for rel, p in enumerate(blks):
            w0 = wpos[g][rel]
            wt[:, w0: w0 + m] = Wp[c0:c0 + m, 128 * p: 128 * (p + 1)].T
    import ml_dtypes
    wt = np.ascontiguousarray(wt.astype(ml_dtypes.bfloat16))

    # ---- x: pad and pre-transpose to [128, 4, U] per batch ----
    j_max = (nblk - 1) // 4
    U = T_out + j_max
    xpad_len = 512 * U
    assert xpad_len >= pad + T, (xpad_len, pad + T)
    xp = np.zeros((B, xpad_len), dtype=np.float32)
    xp[:, pad:pad + T] = x
    # xt[b, r, k*U + u] = xp[b, 512u + 128k + r]
    import ml_dtypes
    xt = np.ascontiguousarray(
        xp.reshape(B, U, 4, 128).transpose(0, 3, 2, 1).reshape(B, 128, 4 * U)
        .astype(ml_dtypes.bfloat16)
    )
    return xt, wt, groups, (wpos, wave_col), C, U, T_out, nbins


def _build_schedule(groups, T_out, b_per):
    """Static balanced 4-slot schedule over the WHOLE core (no per-batch
    barriers).  Jobs are (batch b, group g, t-tile tt); job = len(blocks)
    passes.  Per batch the slot loads are 114/114/114/114; slots flow
    straight from one batch into the next, so PSUM/eviction reuse never
    synchronizes across slots."""
    nts = []
    t0 = 0
    while t0 < T_out:
        nts.append((t0, min(512, T_out - t0)))
        t0 += 512
    assert len(nts) == 3 and len(groups) == 6
    # Slot loads balanced by STREAMING CYCLES (sum of quantum widths), not
    # pass count: the last t-tile is only 268 wide, and the in-order PE
    # queue couples all four tile positions to the slowest slot.  Cycle
    # loads per batch: 49.0k / 49.6k / 50.0k / 47.7k (vs 58.4k max for the
    # pass-balanced schedule).  Job order within a slot starts batch 0
    # from the lowest t-tiles it owns.
    SLOT_JOBS = [
        [(0, 0), (3, 2), (4, 1)],
        [(0, 1), (2, 2)],
        [(1, 0), (4, 0), (5, 0), (3, 1), (5, 1), (0, 2)],
        [(2, 0), (3, 0), (1, 1), (2, 1), (1, 2), (4, 2), (5, 2)],
    ]
    # coverage check
    seen = set()
    for sj in SLOT_JOBS:
        for g, tt in sj:
            assert (g, tt) not in seen
            seen.add((g, tt))
    assert seen == {(g, tt) for g in range(6) for tt in range(3)}

    # flatten to per-slot quanta across all batches
    slot_q = []
    for sj in SLOT_JOBS:
        qs = []
        for b in range(b_per):
            for g, tt in sj:
                blks = groups[g][2]
                for rel, p in enumerate(blks):
                    qs.append(
                        (b, g, tt, p, rel, rel == 0, rel == len(blks) - 1)
                    )
        slot_q.append(qs)

    # merge to one emission list in VIRTUAL-TIME order: always emit for
    # the slot with the least streaming cycles issued so far, so the PE
    # queue (strict in-order issue) interleaves quanta in roughly the
    # order the tile positions actually free up.
    clocks = [0.0] * 4
    idx = [0] * 4
    emit = []
    while any(idx[s] < len(slot_q[s]) for s in range(4)):
        s = min(
            (s for s in range(4) if idx[s] < len(slot_q[s])),
            key=lambda s: (clocks[s], s),
        )
        q = slot_q[s][idx[s]]
        idx[s] += 1
        emit.append((s,) + q)
        clocks[s] += nts[q[2]][1]
    return nts, emit


def _build_program(b_per, C, U, T_out, groups, wmeta):
    import concourse.mybir as mybir
    import concourse.tile as tile
    from concourse import bacc

    f32 = mybir.dt.float32
    bf16 = mybir.dt.bfloat16
    wpos, wave_col = wmeta
    wtot = wave_col[-1]
    nts, emit = _build_schedule(groups, T_out, b_per)

    nc = bacc.Bacc(
        "TRN2",
        target_bir_lowering=False,
        debug=False,
        enable_asserts=True,
        num_devices=N_CORES,
    )
    xt_d = nc.dram_tensor("xt", [b_per, 128, 4 * U], bf16, kind="ExternalInput").ap()
    wt_d = nc.dram_tensor("wt", [128, wtot], bf16, kind="ExternalInput").ap()
    out_d = nc.dram_tensor("out", [b_per, C, T_out], f32, kind="ExternalOutput").ap()

    # weight DMA chunk boundaries: waves 0-23, 24-47, 48+ (large
    # contiguous transfers in consumption order)
    nwave = len(wave_col) - 1
    w_cuts = [0] + [wave_col[min(w, nwave)] for w in (24, 48)] + [wtot]
    w_cuts = sorted(set(w_cuts))

    # x DMA chunk boundaries for batch 0 (u-ranges per t-tile window)
    j_max = (max(groups[0][2])) // 4
    x_stops = []
    for (t0_, nt_) in nts:
        x_stops.append(min(t0_ + nt_ + j_max + 1, U))
    x_stops[-1] = U

    with tile.TileContext(nc) as tc:
        with (
            tc.tile_pool(name="wpool", bufs=1) as wpool,
            tc.tile_pool(name="xpool", bufs=4) as xpool,
            tc.tile_pool(name="evpool", bufs=6) as evpool,
            tc.tile_pool(name="pspool", bufs=8, space="PSUM") as pspool,
        ):
            wsb = wpool.tile([128, wtot], bf16)
            xbs = [
                xpool.tile([128, 4 * U], bf16, tag="xb", name=f"xb{b}")
                for b in range(b_per)
            ]

            # PE warmup: the HAM clock gate keeps the PE at 1.2 GHz until
            # it has been busy ~3.4us.  Burn that in on scratch data while
            # the first input DMAs are in flight so the real matmuls run
            # at 2.4 GHz from pass 0.
            wu_rhs = wpool.tile([128, 512], bf16, name="wu_rhs")
            wu_ps = pspool.tile([128, 512], f32, tag="ps", name="wu_ps")
            nc.vector.memset(wu_rhs[:], 0.0)
            for _ in range(20):
                nc.tensor.matmul(
                    wu_ps[0:32, :],
                    lhsT=wu_rhs[:, 0:32],
                    rhs=wu_rhs[:],
                    start=True,
                    stop=True,
                    tile_position=(0, 0),
                )

            src0 = xt_d[0].rearrange("r (k u) -> r k u", k=4)
            dst0 = xbs[0].rearrange("r (k u) -> r k u", k=4)
            u2 = x_stops[1]
            # scalar queue: weights only, in wave (consumption) order, so
            # the weight stream never starves behind x traffic
            for a0, a1 in zip(w_cuts[:-1], w_cuts[1:]):
                nc.scalar.dma_start(out=wsb[:, a0:a1], in_=wt_d[:, a0:a1])
            # sync queue: batch-0 x windows, the short tail window, then
            # whole-batch prefetches
            nc.sync.dma_start(out=dst0[:, :, 0:u2], in_=src0[:, :, 0:u2])
            nc.sync.dma_start(out=dst0[:, :, u2:U], in_=src0[:, :, u2:U])
            for b in range(1, b_per):
                nc.sync.dma_start(out=xbs[b][:], in_=xt_d[b])

            # one global emission loop in virtual-time order — slots flow
            # across batch boundaries; PSUM banks and eviction buffers
            # rotate through shared pools so reuse distance is ~8 jobs.
            cur_ps = [None] * 4
            for (s, b, g, tt, p, rel, first, last) in emit:
                c0, m, blks = groups[g]
                t0, nt = nts[tt]
                xb = xbs[b]
                if first:
                    cur_ps[s] = pspool.tile(
                        [128, 512], f32, tag="ps", name=f"ps{s}_{b}_{g}_{tt}"
                    )
                ps = cur_ps[s]
                j, k = divmod(p, 4)
                wc = wpos[g][rel]
                nc.tensor.matmul(
                    ps[32 * s: 32 * s + m, :nt],
                    lhsT=wsb[:, wc: wc + m],
                    rhs=xb[:, k * U + t0 + j: k * U + t0 + j + nt],
                    start=first,
                    stop=last,
                    tile_position=(0, 32 * s),
                )
                if last:
                    ev = evpool.tile(
                        [128, 512], f32, tag="ev", name=f"ev{s}_{b}_{g}_{tt}"
                    )
                    if s < 2:
                        nc.vector.tensor_copy(
                            ev[32 * s: 32 * s + m, :nt],
                            ps[32 * s: 32 * s + m, :nt],
                        )
                        nc.sync.dma_start(
                            out=out_d[b, c0:c0 + m, t0:t0 + nt],
                            in_=ev[32 * s: 32 * s + m, :nt],
                        )
                    else:
                        nc.scalar.copy(
                            ev[32 * s: 32 * s + m, :nt],
                            ps[32 * s: 32 * s + m, :nt],
                        )
                        nc.scalar.dma_start(
                            out=out_d[b, c0:c0 + m, t0:t0 + nt],
                            in_=ev[32 * s: 32 * s + m, :nt],
                        )
    nc.compile()
    return nc


def _ensure_trace_shims():
    """If run_bass_kernel_spmd is invoked with tracing enabled (e.g. via
    BASS_TRACE=1) it imports antenv.axon_hooks and uploads artifacts to a
    bucket; neither exists in a bare container.  Register a working NTFF
    hook (ctypes into the axon .so) and a no-op uploader so the trace path
    degrades gracefully instead of crashing."""
    import sys

    try:
        import antenv.axon_hooks  # noqa: F401
    except ImportError:
        import contextlib
        import ctypes
        import types

        hook = None
        try:
            lib = ctypes.CDLL("/opt/axon/libaxon_pjrt.so")
            if hasattr(lib, "axon_start_nrt_profile"):
                lib.axon_start_nrt_profile.argtypes = [
                    ctypes.POINTER(ctypes.c_int64),
                    ctypes.c_size_t,
                ]
                lib.axon_start_nrt_profile.restype = ctypes.c_int64
                lib.axon_stop_nrt_profile.argtypes = [ctypes.c_char_p]
                lib.axon_stop_nrt_profile.restype = ctypes.c_int64

                @contextlib.contextmanager
                def _hook(output_dir, device_ids):
                    import jax

                    jax.devices()
                    if device_ids:
                        ids = (ctypes.c_int64 * len(device_ids))(*device_ids)
                        rc = lib.axon_start_nrt_profile(ids, len(device_ids))
                    else:
                        rc = lib.axon_start_nrt_profile(None, 0)
                    if rc != 0:
                        raise RuntimeError(f"axon_start_nrt_profile rc={rc}")
                    try:
                        yield
                    finally:
                        lib.axon_stop_nrt_profile(str(output_dir).encode())

                hook = _hook
        except OSError:
            pass
        mod = types.ModuleType("antenv.axon_hooks")
        mod.get_axon_ntff_profile_hook = lambda: hook
        mod.set_axon_ntff_profile_hook = lambda h: None
        sys.modules["antenv.axon_hooks"] = mod

    try:
        import concourse.bass_utils as _bu

        _orig_upload = _bu.upload_artifacts

        def _safe_upload(tmpdir):
            try:
                return _orig_upload(tmpdir)
            except Exception:
                return "local://unavailable"

        if not getattr(_bu, "_safe_upload_installed", False):
            _bu.upload_artifacts = _safe_upload
            _bu._safe_upload_installed = True
    except Exception:
        pass


def kernel(x, kernels):
    _ensure_trace_shims()
    from concourse.bass_utils import run_bass_kernel_spmd

    xt, wt, groups, wmeta, C, U, T_out, nbins = _host_prep(x, kernels)
    B = xt.shape[0]
    assert B % N_CORES == 0
    b_per = B // N_CORES

    key = (b_per, C, U, T_out, tuple((c0, m, tuple(b)) for c0, m, b in groups))
    if key not in _prog_cache:
        _prog_cache[key] = _build_program(b_per, C, U, T_out, groups, wmeta)
    nc = _prog_cache[key]

    in_maps = [
        {"xt": xt[c * b_per:(c + 1) * b_per], "wt": wt} for c in range(N_CORES)
    ]
    res = run_bass_kernel_spmd(nc, in_maps, list(range(N_CORES)))
    parts = [res.results[c]["out"] for c in range(N_CORES)]
    out = np.concatenate(parts, axis=0)  # (B, C, T_out)
    return np.ascontiguousarray(
        out.reshape(B, nbins, 2, T_out).transpose(0, 2, 1, 3)
    )


# revision 34
# speedup vs baseline: 1.1816x; 1.0198x over previous
"""CQT (constant-Q transform) kernel for Trainium2, 8 NeuronCores.

Math: out[b, c, t] = sum_l W[c, l] * x_pad[b, t*HOP + l]   (strided conv,
HOP=512, L=11339 taps, C=168 channels = 84 bins x re/im), then reshaped to
(B, 2, n_bins, T_out).

Strategy:
  - Data-parallel: shard B=32 across 8 cores (4 batches/core), weights
    replicated.
  - The conv is decomposed into 128-tap blocks: block p covers taps
    [128p, 128p+128).  The moving operand for block p=(4j+k) at output
    tile [t0, t0+nt) is a contiguous column slice of a host-pre-transposed
    view of x:  xt[r, k, u] = x_pad[512u + 128k + r].
  - CQT kernels are ragged (bin k has ~11339*2^(-k/12) taps, centered), so
    most blocks touch only a few low-bin channels.  A plain matmul costs
    ~N streaming cycles regardless of how few of the 128 PE columns hold
    weights, so the dense-block formulation wastes most of the array.
  - Column tiling: channels are split into groups of 32 (16 bins).  Each
    (block, group) quantum is a K=128, M<=32, N=nt bf16 matmul placed on
    one of the four 32-column PE tile positions (tile_position=(0, 32*s)).
    The 4 tile positions stream concurrently, quartering PE time.
    Quanta per t-tile per group: {89, 36, 15, 7, 3, 2} = 152 vs 92
    full-width matmuls for the dense-block formulation.
  - Jobs (batch, group, t-tile) are assigned to slots balanced by
    STREAMING CYCLES (~49k/slot/batch) and emitted in virtual-time order
    with no batch barriers; PSUM banks (8) and eviction buffers rotate
    through shared pools so reuse never stalls the in-order PE queue.
  - Each job accumulates its blocks into one PSUM bank (per-element
    has_written: first write overwrites, later ones accumulate), then is
    copied psum[32s:32s+m] -> SBUF (vector for slots 0-1, scalar for 2-3)
    and DMA'd out (sync / scalar HW queues).
  - Warmup matmuls on scratch data run during the input DMAs so the HAM
    clock gate is already at 2.4 GHz when the real stream starts; inputs
    arrive as a few large wave-ordered transfers on two HW DMA queues.
"""

import numpy as np

HOP = 512
N_CORES = 8

_prog_cache: dict = {}


def _host_prep(x, kernels):
    x = np.ascontiguousarray(np.asarray(x, dtype=np.float32))
    kernels = np.ascontiguousarray(np.asarray(kernels, dtype=np.float32))
    B, T = x.shape
    nbins, two, Lmax = kernels.shape
    assert two == 2
    C = 2 * nbins
    pad = Lmax // 2
    T_out = (T + 2 * pad - Lmax) // HOP + 1

    # ---- weights: pad taps to 128 multiple ----
    nblk = -(-Lmax // 128)
    Wp = np.zeros((C, nblk * 128), dtype=np.float32)
    Wp[:, :Lmax] = kernels.reshape(C, Lmax)
    nzb = (Wp.reshape(C, nblk, 128) != 0.0).any(axis=2)  # [C, nblk]

    # channel groups of 32 (16 bins); bins are sorted by descending filter
    # length, supports are nested, so a group's active blocks = union over
    # its channels = the blocks of its longest (first) channel.
    groups = []  # (c0, m, blocks)
    for c0 in range(0, C, 32):
        m = min(32, C - c0)
        blks = np.where(nzb[c0:c0 + m].any(axis=0))[0].tolist()
        groups.append((c0, m, blks))

    # Weight layout: per (group, block) a zero-padded [128 taps, m chans]
    # panel.  Panels are laid out in CONSUMPTION order: wave i holds panel
    # i of every group that still has blocks (all slots consume their
    # group's panels in lockstep), so a prefix of wt's columns is exactly
    # the first waves -- weight DMA chunks can be few and large.
    maxlen_b = max(len(b) for _, _, b in groups)
    wpos = [[None] * len(blks) for _, _, blks in groups]
    wave_col = []  # column where wave i starts
    tot = 0
    for i in range(maxlen_b):
        wave_col.append(tot)
        for g, (c0, m, blks) in enumerate(groups):
            if i < len(blks):
                wpos[g][i] = tot
                tot += m
    wave_col.append(tot)
    wt = np.zeros((128, tot), dtype=np.float32)
    for g, (c0, m, blks) in enumerate(groups):
        for rel, p in enumerate(blks):
            w0 = wpos[g][rel]
            wt[:, w0: w0 + m] = Wp[c0:c0 + m, 128 * p: 128 * (p + 1)].T
    import ml_dtypes
    wt = np.ascontiguousarray(wt.astype(ml_dtypes.bfloat16))

    # ---- x: pad and pre-transpose to [128, 4, U] per batch ----
    j_max = (nblk - 1) // 4
    U = T_out + j_max
    xpad_len = 512 * U
    assert xpad_len >= pad + T, (xpad_len, pad + T)
    xp = np.zeros((B, xpad_len), dtype=np.float32)
    xp[:, pad:pad + T] = x
    # xt[b, r, k*U + u] = xp[b, 512u + 128k + r]
    import ml_dtypes
    xt = np.ascontiguousarray(
        xp.reshape(B, U, 4, 128).transpose(0, 3, 2, 1).reshape(B, 128, 4 * U)
        .astype(ml_dtypes.bfloat16)
    )
    return xt, wt, groups, (wpos, wave_col), C, U, T_out, nbins


def _build_schedule(groups, T_out, b_per):
    """Static balanced 4-slot schedule over the WHOLE core (no per-batch
    barriers).  Jobs are (batch b, group g, t-tile tt); job = len(blocks)
    passes.  Per batch the slot loads are 114/114/114/114; slots flow
    straight from one batch into the next, so PSUM/eviction reuse never
    synchronizes across slots."""
    nts = []
    t0 = 0
    while t0 < T_out:
        nts.append((t0, min(512, T_out - t0)))
        t0 += 512
    assert len(nts) == 3 and len(groups) == 6
    # Slot loads balanced by STREAMING CYCLES (sum of quantum widths), not
    # pass count: the last t-tile is only 268 wide, and the in-order PE
    # queue couples all four tile positions to the slowest slot.  Cycle
    # loads per batch: 49.0k / 49.6k / 50.0k / 47.7k (vs 58.4k max for the
    # pass-balanced schedule).  Job order within a slot starts batch 0
    # from the lowest t-tiles it owns.
    SLOT_JOBS = [
        [(0, 0), (3, 2), (4, 1)],
        [(0, 1), (2, 2)],
        [(1, 0), (4, 0), (5, 0), (3, 1), (5, 1), (0, 2)],
        [(2, 0), (3, 0), (1, 1), (2, 1), (1, 2), (4, 2), (5, 2)],
    ]
    # coverage check
    seen = set()
    for sj in SLOT_JOBS:
        for g, tt in sj:
            assert (g, tt) not in seen
            seen.add((g, tt))
    assert seen == {(g, tt) for g in range(6) for tt in range(3)}

    # flatten to per-slot quanta across all batches
    slot_q = []
    for sj in SLOT_JOBS:
        qs = []
        for b in range(b_per):
            for g, tt in sj:
                blks = groups[g][2]
                for rel, p in enumerate(blks):
                    qs.append(
                        (b, g, tt, p, rel, rel == 0, rel == len(blks) - 1)
                    )
        slot_q.append(qs)

    # merge to one emission list in VIRTUAL-TIME order: always emit for
    # the slot with the least streaming cycles issued so far, so the PE
    # queue (strict in-order issue) interleaves quanta in roughly the
    # order the tile positions actually free up.
    clocks = [0.0] * 4
    idx = [0] * 4
    emit = []
    while any(idx[s] < len(slot_q[s]) for s in range(4)):
        s = min(
            (s for s in range(4) if idx[s] < len(slot_q[s])),
            key=lambda s: (clocks[s], s),
        )
        q = slot_q[s][idx[s]]
        idx[s] += 1
        emit.append((s,) + q)
        clocks[s] += nts[q[2]][1]
    return nts, emit


def _build_program(b_per, C, U, T_out, groups, wmeta):
    import concourse.mybir as mybir
    import concourse.tile as tile
    from concourse import bacc

    f32 = mybir.dt.float32
    bf16 = mybir.dt.bfloat16
    wpos, wave_col = wmeta
    wtot = wave_col[-1]
    nts, emit = _build_schedule(groups, T_out, b_per)

    nc = bacc.Bacc(
        "TRN2",
        target_bir_lowering=False,
        debug=False,
        enable_asserts=True,
        num_devices=N_CORES,
    )
    xt_d = nc.dram_tensor("xt", [b_per, 128, 4 * U], bf16, kind="ExternalInput").ap()
    wt_d = nc.dram_tensor("wt", [128, wtot], bf16, kind="ExternalInput").ap()
    out_d = nc.dram_tensor("out", [b_per, C, T_out], f32, kind="ExternalOutput").ap()

    # weight DMA chunk boundaries: waves 0-23, 24-47, 48+ (large
    # contiguous transfers in consumption order)
    nwave = len(wave_col) - 1
    w_cuts = [0] + [wave_col[min(w, nwave)] for w in (24, 48)] + [wtot]
    w_cuts = sorted(set(w_cuts))

    # x DMA chunk boundaries for batch 0 (u-ranges per t-tile window)
    j_max = (max(groups[0][2])) // 4
    x_stops = []
    for (t0_, nt_) in nts:
        x_stops.append(min(t0_ + nt_ + j_max + 1, U))
    x_stops[-1] = U

    with tile.TileContext(nc) as tc:
        with (
            tc.tile_pool(name="wpool", bufs=1) as wpool,
            tc.tile_pool(name="xpool", bufs=4) as xpool,
            tc.tile_pool(name="evpool", bufs=6) as evpool,
            tc.tile_pool(name="pspool", bufs=8, space="PSUM") as pspool,
        ):
            wsb = wpool.tile([128, wtot], bf16)
            xbs = [
                xpool.tile([128, 4 * U], bf16, tag="xb", name=f"xb{b}")
                for b in range(b_per)
            ]

            # PE warmup: the HAM clock gate keeps the PE at 1.2 GHz until
            # it has been busy ~3.4us.  Burn that in on scratch data while
            # the first input DMAs are in flight so the real matmuls run
            # at 2.4 GHz from pass 0.
            wu_rhs = wpool.tile([128, 512], bf16, name="wu_rhs")
            wu_ps = pspool.tile([128, 512], f32, tag="ps", name="wu_ps")
            nc.vector.memset(wu_rhs[:], 0.0)
            for _ in range(20):
                nc.tensor.matmul(
                    wu_ps[0:32, :],
                    lhsT=wu_rhs[:, 0:32],
                    rhs=wu_rhs[:],
                    start=True,
                    stop=True,
                    tile_position=(0, 0),
                )

            src0 = xt_d[0].rearrange("r (k u) -> r k u", k=4)
            dst0 = xbs[0].rearrange("r (k u) -> r k u", k=4)
            u2 = x_stops[1]
            # scalar queue: first weight wave chunk, x k-planes 2-3, then
            # the remaining weight chunks (all large transfers)
            nc.scalar.dma_start(
                out=wsb[:, w_cuts[0]:w_cuts[1]], in_=wt_d[:, w_cuts[0]:w_cuts[1]]
            )
            nc.scalar.dma_start(out=dst0[:, 2:4, 0:u2], in_=src0[:, 2:4, 0:u2])
            for a0, a1 in zip(w_cuts[1:-1], w_cuts[2:]):
                nc.scalar.dma_start(out=wsb[:, a0:a1], in_=wt_d[:, a0:a1])
            # sync queue: x k-planes 0-1, the short tail window, then
            # whole-batch prefetches
            nc.sync.dma_start(out=dst0[:, 0:2, 0:u2], in_=src0[:, 0:2, 0:u2])
            nc.sync.dma_start(out=dst0[:, :, u2:U], in_=src0[:, :, u2:U])
            for b in range(1, b_per):
                nc.sync.dma_start(out=xbs[b][:], in_=xt_d[b])

            # one global emission loop in virtual-time order — slots flow
            # across batch boundaries; PSUM banks and eviction buffers
            # rotate through shared pools so reuse distance is ~8 jobs.
            cur_ps = [None] * 4
            for (s, b, g, tt, p, rel, first, last) in emit:
                c0, m, blks = groups[g]
                t0, nt = nts[tt]
                xb = xbs[b]
                if first:
                    cur_ps[s] = pspool.tile(
                        [128, 512], f32, tag="ps", name=f"ps{s}_{b}_{g}_{tt}"
                    )
                ps = cur_ps[s]
                j, k = divmod(p, 4)
                wc = wpos[g][rel]
                nc.tensor.matmul(
                    ps[32 * s: 32 * s + m, :nt],
                    lhsT=wsb[:, wc: wc + m],
                    rhs=xb[:, k * U + t0 + j: k * U + t0 + j + nt],
                    start=first,
                    stop=last,
                    tile_position=(0, 32 * s),
                )
                if last:
                    ev = evpool.tile(
                        [128, 512], f32, tag="ev", name=f"ev{s}_{b}_{g}_{tt}"
                    )
                    if s < 2:
                        nc.vector.tensor_copy(
                            ev[32 * s: 32 * s + m, :nt],
                            ps[32 * s: 32 * s + m, :nt],
                        )
                        nc.sync.dma_start(
                            out=out_d[b, c0:c0 + m, t0:t0 + nt],
                            in_=ev[32 * s: 32 * s + m, :nt],
                        )
                    else:
                        nc.scalar.copy(
                            ev[32 * s: 32 * s + m, :nt],
                            ps[32 * s: 32 * s + m, :nt],
                        )
                        nc.scalar.dma_start(
                            out=out_d[b, c0:c0 + m, t0:t0 + nt],
                            in_=ev[32 * s: 32 * s + m, :nt],
                        )
    nc.compile()
    return nc


def _ensure_trace_shims():
    """If run_bass_kernel_spmd is invoked with tracing enabled (e.g. via
    BASS_TRACE=1) it imports antenv.axon_hooks and uploads artifacts to a
    bucket; neither exists in a bare container.  Register a working NTFF
    hook (ctypes into the axon .so) and a no-op uploader so the trace path
    degrades gracefully instead of crashing."""
    import sys

    try:
        import antenv.axon_hooks  # noqa: F401
    except ImportError:
        import contextlib
        import ctypes
        import types

        hook = None
        try:
            lib = ctypes.CDLL("/opt/axon/libaxon_pjrt.so")
            if hasattr(lib, "axon_start_nrt_profile"):
                lib.axon_start_nrt_profile.argtypes = [
                    ctypes.POINTER(ctypes.c_int64),
                    ctypes.c_size_t,
                ]
                lib.axon_start_nrt_profile.restype = ctypes.c_int64
                lib.axon_stop_nrt_profile.argtypes = [ctypes.c_char_p]
                lib.axon_stop_nrt_profile.restype = ctypes.c_int64

                @contextlib.contextmanager
                def _hook(output_dir, device_ids):
                    import jax

                    jax.devices()
                    if device_ids:
                        ids = (ctypes.c_int64 * len(device_ids))(*device_ids)
                        rc = lib.axon_start_nrt_profile(ids, len(device_ids))
                    else:
                        rc = lib.axon_start_nrt_profile(None, 0)
                    if rc != 0:
                        raise RuntimeError(f"axon_start_nrt_profile rc={rc}")
                    try:
                        yield
                    finally:
                        lib.axon_stop_nrt_profile(str(output_dir).encode())

                hook = _hook
        except OSError:
            pass
        mod = types.ModuleType("antenv.axon_hooks")
        mod.get_axon_ntff_profile_hook = lambda: hook
        mod.set_axon_ntff_profile_hook = lambda h: None
        sys.modules["antenv.axon_hooks"] = mod

    try:
        import concourse.bass_utils as _bu

        _orig_upload = _bu.upload_artifacts

        def _safe_upload(tmpdir):
            try:
                return _orig_upload(tmpdir)
            except Exception:
                return "local://unavailable"

        if not getattr(_bu, "_safe_upload_installed", False):
            _bu.upload_artifacts = _safe_upload
            _bu._safe_upload_installed = True
    except Exception:
        pass


def kernel(x, kernels):
    _ensure_trace_shims()
    from concourse.bass_utils import run_bass_kernel_spmd

    xt, wt, groups, wmeta, C, U, T_out, nbins = _host_prep(x, kernels)
    B = xt.shape[0]
    assert B % N_CORES == 0
    b_per = B // N_CORES

    key = (b_per, C, U, T_out, tuple((c0, m, tuple(b)) for c0, m, b in groups))
    if key not in _prog_cache:
        _prog_cache[key] = _build_program(b_per, C, U, T_out, groups, wmeta)
    nc = _prog_cache[key]

    in_maps = [
        {"xt": xt[c * b_per:(c + 1) * b_per], "wt": wt} for c in range(N_CORES)
    ]
    res = run_bass_kernel_spmd(nc, in_maps, list(range(N_CORES)))
    parts = [res.results[c]["out"] for c in range(N_CORES)]
    out = np.concatenate(parts, axis=0)  # (B, C, T_out)
    return np.ascontiguousarray(
        out.reshape(B, nbins, 2, T_out).transpose(0, 2, 1, 3)
    )
